# revision 1
# baseline (speedup 1.0000x reference)
"""AlignmentAttentionLayer Trainium2 kernel (8 NeuronCores, data-parallel).

Math per batch row b (D=300, L=50):
    M     = tanh(W_y @ Y[b] + (W_h @ h_n[b]) 1_L^T)     [D, L]
    alpha = softmax(w^T M)  over L                       [L]
    r     = Y[b] @ alpha                                 [D]
    out   = tanh(W_p @ r + W_x @ h_n[b])                 [D]

Sharding: batch dim B=16384 split across 8 cores (2048 rows each);
weights replicated. No collectives needed.

Layout strategy per core:
  - Y chunk in SBUF as [e(partitions, 3 subtiles of 128/128/44), b, l],
    cast once to bf16 (PE + DVE run faster on bf16).
  - All D x D weights preloaded transposed [e, d] as bf16.
  - M = tanh(W_y Y + bias) computed per 400-column chunk in PSUM
    ([d_sub, (b,l)]), bias added on DVE, tanh on ACT, output bf16.
  - s = w^T M via PE matmul with lhsT = w replicated to 128 columns,
    so exp(s) lands replicated across all 128 partitions, which makes
    the alpha * Y elementwise product / row-reduction partition-parallel.
  - softmax normalization folded into r: r = (Y @ exp(s)) / sum(exp(s)).
  - h* accumulated in PSUM from 6 matmuls (W_p r + W_x h_n), tanh,
    PE-transposed back to [b, d] and DMA'd out contiguously.
"""

import numpy as np

# ---- constants (hardcoded per problem spec) -------------------------------
B = 16384
D = 300
L = 50
NCORES = 8
BB = B // NCORES          # 2048 batch rows per core
P = 128
SUBS = [(0, 128), (128, 128), (256, 44)]   # subtiles of the 300-dim axis
NB = 64                   # batch rows per outer chunk
NCH = 8                   # inner column-chunks per outer chunk
NI = NB * L // NCH        # 400 (b,l) columns per inner chunk
RI = NI // L              # 8 batch rows per inner chunk


def _build(bb: int):
    import concourse.bass as bass
    import concourse.mybir as mybir
    from concourse.tile import TileContext
    from concourse.masks import make_identity

    f32 = mybir.dt.float32
    bf16 = mybir.dt.bfloat16
    AF = mybir.ActivationFunctionType
    OP = mybir.AluOpType
    AX = mybir.AxisListType

    from contextlib import ExitStack

    nc = bass.Bass("TRN2")
    Y_d = nc.declare_dram_parameter("Y", [bb, D, L], f32, isOutput=False)
    hn_d = nc.declare_dram_parameter("h_n", [bb, D], f32, isOutput=False)
    Wy_d = nc.declare_dram_parameter("W_y", [D, D], f32, isOutput=False)
    Wh_d = nc.declare_dram_parameter("W_h", [D, D], f32, isOutput=False)
    Wp_d = nc.declare_dram_parameter("W_p", [D, D], f32, isOutput=False)
    Wx_d = nc.declare_dram_parameter("W_x", [D, D], f32, isOutput=False)
    w_d = nc.declare_dram_parameter("w", [D], f32, isOutput=False)
    out_d = nc.declare_dram_parameter("out", [bb, D], f32, isOutput=True)

    chunks = bb // NB

    with TileContext(nc) as tc, ExitStack() as ctx:
        const = ctx.enter_context(tc.tile_pool(name="const", bufs=1))
        init = ctx.enter_context(tc.tile_pool(name="init", bufs=2))
        work = ctx.enter_context(tc.tile_pool(name="work", bufs=2))
        ypool = ctx.enter_context(tc.tile_pool(name="ypool", bufs=3))
        inner = ctx.enter_context(tc.tile_pool(name="inner", bufs=4))
        psM = ctx.enter_context(tc.tile_pool(name="psM", bufs=3, space="PSUM"))
        psS = ctx.enter_context(tc.tile_pool(name="psS", bufs=2, space="PSUM"))
        psA = ctx.enter_context(tc.tile_pool(name="psA", bufs=3, space="PSUM"))

        ident = const.tile([P, P], f32, tag="ident")
        make_identity(nc, ident)

        # ---- preload weights, transposed [e, d], bf16 ----
        wTs = {}
        for name, wd in (("wy", Wy_d), ("wh", Wh_d), ("wp", Wp_d), ("wx", Wx_d)):
            wT = const.tile([P, 3, D], bf16, tag=f"{name}T")
            tmp = init.tile([P, 3, D], f32, tag="wtmp")
            with nc.allow_non_contiguous_dma(reason="one-time 300x300 transpose load"):
                for es, (e0, pe) in enumerate(SUBS):
                    nc.scalar.dma_start(out=tmp[:pe, es, :], in_=wd[:, e0:e0 + pe].rearrange("d e -> e d"))
            for es, (e0, pe) in enumerate(SUBS):
                nc.gpsimd.tensor_copy(out=wT[:pe, es, :], in_=tmp[:pe, es, :])
            wTs[name] = wT
        wyT, whT, wpT, wxT = wTs["wy"], wTs["wh"], wTs["wp"], wTs["wx"]

        # ---- w replicated to 128 columns: lhsT for the s-matmul ----
        wv = const.tile([P, 3], f32, tag="wv")
        with nc.allow_non_contiguous_dma(reason="one-time 300-elem strided load"):
            for es, (e0, pe) in enumerate(SUBS):
                nc.scalar.dma_start(out=wv[:pe, es:es + 1], in_=w_d[e0:e0 + pe, None])
        w_repl = const.tile([P, 3, P], bf16, tag="w_repl")
        for es, (e0, pe) in enumerate(SUBS):
            nc.vector.tensor_copy(out=w_repl[:pe, es, :], in_=wv[:pe, es, None].to_broadcast((pe, P)))

        # ---- bf16 identity; broadcast over l at matmul time it acts as the
        # block-indicator mask for the PE bias-inject matmul:
        # psum[d, (j,l)] += sum_b whn[b, d] * I[b, r0+j] = wh[d, b=r0+j].
        identb = const.tile([NB, NB], bf16, tag="identb")
        nc.vector.tensor_copy(out=identb[:], in_=ident[:NB, :NB])

        # ---- main loop over batch chunks ----
        for c in range(chunks):
            b0 = c * NB

            Yf = ypool.tile([P, 3, NB, L], f32, tag="Yf")
            H = NB // 2
            dma_eng = {(0, 0): nc.sync, (0, 1): nc.sync, (1, 0): nc.sync,
                       (1, 1): nc.scalar, (2, 0): nc.gpsimd, (2, 1): nc.gpsimd}
            for es, (e0, pe) in enumerate(SUBS):
                for h in range(2):
                    dma_eng[(es, h)].dma_start(
                        out=Yf[:pe, es, h * H:(h + 1) * H],
                        in_=Y_d[b0 + h * H:b0 + (h + 1) * H, e0:e0 + pe, :].rearrange("b e l -> e b l"),
                    )
            Yb = work.tile([P, 3, NB, L], bf16, tag="Yb")
            cast_eng = [nc.vector, nc.vector, nc.gpsimd]
            for es, (e0, pe) in enumerate(SUBS):
                cast_eng[es].tensor_copy(out=Yb[:pe, es], in_=Yf[:pe, es])

            hn = work.tile([NB, D], f32, tag="hn")
            nc.gpsimd.dma_start(out=hn[:], in_=hn_d[b0:b0 + NB, :])
            hnT = work.tile([P, 3, NB], bf16, tag="hnT")
            for es, (e0, pe) in enumerate(SUBS):
                pt = psA.tile([P, P], f32, tag="psa", name="pt")[:, :NB]
                nc.tensor.transpose(pt[:pe, :NB], hn[:, e0:e0 + pe], ident[:NB, :NB])
                nc.scalar.copy(out=hnT[:pe, es, :], in_=pt[:pe, :NB])

            # bias in natural orientation: whn[b, d] = h_n @ W_h^T
            whn = work.tile([NB, D], bf16, tag="whn")
            pwhn = psA.tile([P, D], f32, tag="psa", name="pwhn")[:NB, :]
            for es, (e0, pe) in enumerate(SUBS):
                nc.tensor.matmul(pwhn[:, :], hnT[:pe, es, :], whT[:pe, es, :],
                                 start=(es == 0), stop=(es == 2))
            nc.scalar.copy(out=whn[:], in_=pwhn[:])

            rT = work.tile([P, 3, NB], f32, tag="rT")
            z = work.tile([P, NB], f32, tag="z")

            for t in range(NCH):
                r0 = t * RI
                Mb = inner.tile([P, 3, NI], bf16, tag="Mb")
                for ds, (d0, pd) in enumerate(SUBS):
                    pm = psM.tile([P, NI], f32, tag="pm")
                    for es, (e0, pe) in enumerate(SUBS):
                        nc.tensor.matmul(
                            pm[:pd, :], wyT[:pe, es, d0:d0 + pd],
                            Yb[:pe, es, r0:r0 + RI, :], start=(es == 0), stop=False)
                    nc.tensor.matmul(
                        pm[:pd, :], whn[:, d0:d0 + pd],
                        identb[:, r0:r0 + RI, None].to_broadcast((NB, RI, L)),
                        start=False, stop=True)
                    nc.scalar.activation(out=Mb[:pd, ds], in_=pm[:pd, :], func=AF.Tanh)

                ps_s = psS.tile([P, NI], f32, tag="ps_s")
                for ds, (d0, pd) in enumerate(SUBS):
                    nc.tensor.matmul(ps_s[:, :], w_repl[:pd, ds, :], Mb[:pd, ds],
                                     start=(ds == 0), stop=(ds == 2))
                alpha = inner.tile([P, NI], bf16, tag="alpha")
                nc.scalar.activation(out=alpha[:], in_=ps_s[:], func=AF.Exp)
                nc.vector.tensor_reduce(
                    out=z[:, r0:r0 + RI],
                    in_=alpha.rearrange("p (b l) -> p b l", l=L),
                    axis=AX.X, op=OP.add)
                for es, (e0, pe) in enumerate(SUBS):
                    prod = inner.tile([P, RI, L], bf16, tag="prod")
                    prod_eng = nc.gpsimd if es == 2 else nc.vector
                    prod_eng.tensor_mul(
                        out=prod[:pe],
                        in0=Yb[:pe, es, r0:r0 + RI, :],
                        in1=alpha[:pe].rearrange("p (b l) -> p b l", l=L))
                    nc.vector.tensor_reduce(
                        out=rT[:pe, es, r0:r0 + RI], in_=prod[:pe],
                        axis=AX.X, op=OP.add)

            zinv = work.tile([P, NB], f32, tag="zinv")
            nc.vector.reciprocal(zinv[:], z[:])
            rTb = work.tile([P, 3, NB], bf16, tag="rTb")
            for es, (e0, pe) in enumerate(SUBS):
                nc.vector.tensor_mul(out=rTb[:pe, es], in0=rT[:pe, es],
                                     in1=zinv[:pe, :])

            ho = work.tile([NB, D], f32, tag="ho")
            for ds, (d0, pd) in enumerate(SUBS):
                ph = psA.tile([P, P], f32, tag="psa", name="ph")[:, :NB]
                for es, (e0, pe) in enumerate(SUBS):
                    nc.tensor.matmul(ph[:pd, :], wpT[:pe, es, d0:d0 + pd], rTb[:pe, es, :],
                                     start=(es == 0), stop=False)
                for es, (e0, pe) in enumerate(SUBS):
                    nc.tensor.matmul(ph[:pd, :], wxT[:pe, es, d0:d0 + pd], hnT[:pe, es, :],
                                     start=False, stop=(es == 2))
                hs = work.tile([P, NB], f32, tag="hs")
                nc.scalar.activation(out=hs[:pd, :], in_=ph[:pd, :], func=AF.Tanh)
                pt2 = psA.tile([P, P], f32, tag="psa", name="pt2")
                nc.tensor.transpose(pt2[:NB, :pd], hs[:pd, :NB], ident[:pd, :pd])
                nc.scalar.copy(out=ho[:, d0:d0 + pd], in_=pt2[:NB, :pd])

            nc.scalar.dma_start(out=out_d[b0:b0 + NB, :], in_=ho[:])

    return nc


_NC_CACHE = {}


def _install_walrus_workarounds():
    """This container's walrus build supports only ONE semaphore wait per
    instruction, but Tile attaches several (end-of-kernel drain, and any
    body instruction waiting on multiple producers). Two patches:
      1. TileContext._drain_and_barrier: split drain waits into single-wait
         nops on the SP engine (same engine => program order preserved).
      2. compile_bir_kernel: post-process the BIR JSON, inserting a
         single-wait NoOp before any instruction carrying >1 wait.
    """
    import json as _json
    import concourse.mybir as mybir
    import concourse.tile as ctile
    from concourse.tile import ScopedClock
    from concourse import bass_utils, bass2jax

    def _patched_drain_and_barrier(self, tick_clock, wait_clock):
        nc = self.nc
        collector = nc.sync.nop(nofuse=True)
        wait_clock.add_sem_waits(
            collector.ins, ScopedClock({None: tick_clock.global_clock}))
        si = collector.ins.sync_info
        waits = list(si.on_wait) if si is not None else []
        if len(waits) > 1:
            collector.ins.sync_info = mybir.SyncInfo(
                on_wait=[waits[0]], on_update=list(si.on_update))
            for w in waits[1:]:
                n = nc.sync.nop(nofuse=True)
                n.ins.sync_info = mybir.SyncInfo(on_wait=[w], on_update=[])
        nc.sync.drain()
        nc.all_engine_barrier()
        popped = nc._tile_sem_poison_stack.pop()
        assert popped is self._sem_poison
        nc.clear_and_free_semaphores(list(self.sems.allocated().values()))
        nc.all_engine_barrier()

    ctile.TileContext._drain_and_barrier = _patched_drain_and_barrier

    if getattr(bass_utils.compile_bir_kernel, "_wsplit_wrapped", False):
        return
    counter = [0]

    def _split_multiwait_bir(bir_json):
        bir = _json.loads(bir_json)
        changed = False
        for func in bir.get("functions", []):
            for blk in func.get("blocks", []):
                insts = blk.get("instructions")
                if not insts:
                    continue
                out = []
                for ins in insts:
                    si = ins.get("sync_info")
                    waits = (si or {}).get("on_wait") or []
                    if len(waits) > 1:
                        changed = True
                        for w in waits[:-1]:
                            counter[0] += 1
                            out.append({
                                "debug": ins.get("debug"),
                                "engine": ins["engine"],
                                "ins": [], "outs": [],
                                "name": f"I-wsplit-{counter[0]}",
                                "opcode": "NoOp",
                                "sync_info": {"on_update": [], "on_wait": [w]},
                            })
                        si["on_wait"] = [waits[-1]]
                    out.append(ins)
                blk["instructions"] = out
        return _json.dumps(bir).encode() if changed else bir_json

    _orig_compile = bass_utils.compile_bir_kernel

    def compile_bir_kernel(bir_json, tmpdir, neff_name="file.neff"):
        return _orig_compile(_split_multiwait_bir(bir_json), tmpdir, neff_name)

    compile_bir_kernel._wsplit_wrapped = True
    bass_utils.compile_bir_kernel = compile_bir_kernel
    bass2jax.compile_bir_kernel = compile_bir_kernel


def _get_nc(bb: int):
    if bb not in _NC_CACHE:
        _install_walrus_workarounds()
        _NC_CACHE[bb] = _build(bb)
    return _NC_CACHE[bb]


def kernel(Y, h_n, W_y, W_h, W_p, W_x, w, _collect=None):
    from concourse.bass_utils import run_bass_kernel_spmd

    Y = np.ascontiguousarray(np.asarray(Y, dtype=np.float32))
    h_n = np.ascontiguousarray(np.asarray(h_n, dtype=np.float32))
    W_y = np.ascontiguousarray(np.asarray(W_y, dtype=np.float32))
    W_h = np.ascontiguousarray(np.asarray(W_h, dtype=np.float32))
    W_p = np.ascontiguousarray(np.asarray(W_p, dtype=np.float32))
    W_x = np.ascontiguousarray(np.asarray(W_x, dtype=np.float32))
    w = np.ascontiguousarray(np.asarray(w, dtype=np.float32))

    bb = Y.shape[0] // NCORES
    nc = _get_nc(bb)
    in_maps = [
        {
            "Y": Y[i * bb:(i + 1) * bb],
            "h_n": h_n[i * bb:(i + 1) * bb],
            "W_y": W_y, "W_h": W_h, "W_p": W_p, "W_x": W_x, "w": w,
        }
        for i in range(NCORES)
    ]
    res = run_bass_kernel_spmd(nc, in_maps, core_ids=list(range(NCORES)))
    if _collect is not None:
        _collect.append(res)
    return np.concatenate([res.results[i]["out"] for i in range(NCORES)], axis=0)

